# revision 10
# baseline (speedup 1.0000x reference)
"""LlamaMoE (H=2048, I=4096, E=8 experts, top-2, N=2048 tokens) on 8 trn2 cores.

Strategy: expert-parallel with sparse token dispatch. Core c owns expert c
and computes it only on the ~C tokens routed to it (host supplies the
dispatch permutation = token indices per expert, padded with OOB sentinels;
all model math — router logits, top-2 combine weights, expert MLPs, base
MLP, combine — runs on device). Base MLP is column-sharded 1/8 per core.

Router is token-sharded: each core computes fp32 logits + top-2 combine
weights for its 256-token shard (all 8 expert columns), then a small
AllGather replicates the full [2048, 8] combine table; each core extracts
its expert's column for its dispatched tokens via indirect gather + dot
with a one-hot. Expert rows are combined into the dense base rows with a
single scatter-ADD DMA (gpsimd software DGE accumulate) per token chunk.

Order: router -> base mm1 (4 double-buffered token passes) -> gather +
transpose dispatched tokens -> expert mm1 -> per column quarter (base
down first so its DRAM writes are in flight, then expert down with fused
scale + scatter-add, then f16 ReduceScatter). Each core returns a
disjoint f16 row shard which the host concatenates and casts.
"""

import numpy as np

import concourse.bacc as bacc
import concourse.bass as bass
import concourse.mybir as mybir
import concourse.tile as tile
from concourse.bass_utils import run_bass_kernel_spmd
from concourse.masks import make_identity

P = 128
H = 2048
I_EXP = 4096
E = 8
NCORE = 8
NTOK = 2048
NSH = NTOK // NCORE         # 256 router-shard tokens per core
TBS = NSH // P              # 2 router token blocks
KO = H // P                 # 16 contraction tiles for mm1
IC_E = I_EXP // P           # 32 expert intermediate chunks
IC_B = (I_EXP // NCORE) // P  # 4 base-shard chunks per core
ICT = IC_E + IC_B           # 36 contraction tiles for mm2
NPASS = 4                   # token passes for the base MLP
TPASS = NTOK // NPASS       # 512
NB1 = 512                   # mm1 moving free dim (tokens)
HN = 256                    # mm2 moving free dim (H cols)
HNC = H // HN               # 8
NQ = 8                      # column slices for combine/RS pipelining
HQ = H // NQ                # 256
TBF = NTOK // P             # 16 token blocks (full)

F32 = mybir.dt.float32
F16 = mybir.dt.float16
I32 = mybir.dt.int32
AF = mybir.ActivationFunctionType
ALU = mybir.AluOpType
AXX = mybir.AxisListType.X

OOB_IDX = 1 << 20


def _chunks(total, step):
    out = []
    o = 0
    while o < total:
        out.append((o, min(step, total - o)))
        o += step
    return out


def _build(C):
    NTC = C // P  # gathered token chunks
    nc = bacc.Bacc(None)
    xt16_d = nc.dram_tensor("xt16", [P, KO, NTOK], F16, kind="ExternalInput")
    xt32_d = nc.dram_tensor("xt32", [P, KO, NSH], F32, kind="ExternalInput")
    xrow_d = nc.dram_tensor("xrow", [NTOK, H], F16, kind="ExternalInput")
    wgu_d = nc.dram_tensor("wgu", [P, ICT, KO, 2 * P], F16, kind="ExternalInput")
    wd_d = nc.dram_tensor("wd", [P, HNC, ICT, HN], F16, kind="ExternalInput")
    gw_d = nc.dram_tensor("gw", [P, KO, E], F32, kind="ExternalInput")
    esel_d = nc.dram_tensor("esel", [P, E], F32, kind="ExternalInput")
    tidx_d = nc.dram_tensor("tidx", [P, NTC], I32, kind="ExternalInput")
    out_d = nc.dram_tensor("out", [NQ, NTOK // NCORE, HQ], F16, kind="ExternalOutput")

    with tile.TileContext(nc) as tc:
        with (
            tc.tile_pool(name="persist", bufs=1) as persist,
            tc.tile_pool(name="xt", bufs=2) as xtp,
            tc.tile_pool(name="ht", bufs=1) as htp,
            tc.tile_pool(name="wgup", bufs=2) as wgup,
            tc.tile_pool(name="wdp", bufs=4) as wdp,
            tc.tile_pool(name="xk32", bufs=2) as xk32p,
            tc.tile_pool(name="xgp", bufs=3) as xgp,
            tc.tile_pool(name="tmp", bufs=3) as tmpp,
            tc.tile_pool(name="yst", bufs=6) as ystp,
            tc.tile_pool(name="rmw", bufs=3) as rmwp,
            tc.tile_pool(name="rsm", bufs=1) as rsm,
            tc.tile_pool(name="ps1", bufs=2, space="PSUM") as ps1,
            tc.tile_pool(name="ps2", bufs=2, space="PSUM") as ps2,
            tc.tile_pool(name="dram", bufs=1, space="DRAM") as dram,
        ):
            ident = persist.tile([P, P], F32, tag="ident")
            make_identity(nc, ident)
            identf = persist.tile([P, P], F16, tag="identf")
            make_identity(nc, identf)
            gw_sb = persist.tile([P, KO, E], F32, tag="gw")
            nc.sync.dma_start(gw_sb, gw_d[:])
            esel_sb = persist.tile([P, E], F32, tag="esel")
            nc.sync.dma_start(esel_sb, esel_d[:])
            idx_sb = persist.tile([P, NTC], I32, tag="idx")
            nc.sync.dma_start(idx_sb, tidx_d[:])

            rs_half = [
                dram.tile([NTOK, HQ], F16, tag=f"rsin{h_}", name=f"rsin{h_}")
                for h_ in range(NQ)
            ]
            rs_out = [
                dram.tile(
                    [NTOK // NCORE, HQ], F16, tag=f"rsout{h_}", name=f"rsout{h_}"
                )
                for h_ in range(NQ)
            ]
            comb_shard = dram.tile([NSH, E], F32, tag="combsh", name="combsh")
            comb_all = dram.tile([NTOK, E], F32, tag="comball", name="comball")

            # ============ router on this core's 256-token shard (fp32) =====
            lg_ps = ps1.tile([E, NSH], F32, tag="pg", name="lg")
            for k in range(KO):
                xk = xk32p.tile([P, NSH], F32, tag="xk")
                nc.sync.dma_start(xk, xt32_d[:, k, :])
                nc.tensor.matmul(
                    lg_ps, gw_sb[:, k, :], xk,
                    start=(k == 0), stop=(k == KO - 1),
                )
            lgt = rsm.tile([E, NSH], F32, tag="lgt")
            nc.vector.tensor_copy(lgt, lg_ps)
            zl = rsm.tile([P, TBS, E], F32, tag="zl")
            for tb in range(TBS):
                pt = ps2.tile([P, HN], F32, tag="py", bufs=3, name=f"pt{tb}")
                nc.tensor.transpose(
                    pt[:, :E], lgt[:, tb * P:(tb + 1) * P], ident[:E, :E]
                )
                nc.vector.tensor_copy(zl[:, tb, :], pt[:, :E])
            lmax = rsm.tile([P, TBS], F32, tag="lmax")
            nc.vector.reduce_max(lmax[:, :, None], zl, axis=AXX)
            nmax = rsm.tile([P, TBS], F32, tag="nmax")
            nc.vector.tensor_scalar_mul(nmax, lmax, -1.0)
            zex = rsm.tile([P, TBS, E], F32, tag="zex")
            for tb in range(TBS):
                nc.scalar.activation(
                    zex[:, tb, :], zl[:, tb, :], AF.Exp, bias=nmax[:, tb:tb + 1]
                )
            zlt = rsm.tile([P, TBS, E], F32, tag="zlt")
            nc.vector.tensor_scalar(zlt, zex, 1.0, None, op0=ALU.is_lt)
            zmk = rsm.tile([P, TBS, E], F32, tag="zmk")
            nc.vector.tensor_tensor(zmk, zex, zlt, ALU.mult)
            m2 = rsm.tile([P, TBS], F32, tag="m2")
            nc.vector.reduce_max(m2[:, :, None], zmk, axis=AXX)
            s1 = rsm.tile([P, TBS], F32, tag="s1")
            nc.vector.tensor_scalar_add(s1, m2, 1.0)
            rcp = rsm.tile([P, TBS], F32, tag="rcp")
            nc.vector.reciprocal(rcp, s1)
            # combine weight for every expert: zex * (zex >= m2) / (1 + m2)
            cwn = rsm.tile([P, TBS, E], F32, tag="cwn")
            for tb in range(TBS):
                ge = rsm.tile([P, E], F32, tag="ge")
                nc.vector.tensor_scalar(
                    ge, zex[:, tb, :], m2[:, tb:tb + 1], None, op0=ALU.is_ge
                )
                cw = rsm.tile([P, E], F32, tag="cw")
                nc.vector.tensor_tensor(cw, zex[:, tb, :], ge, ALU.mult)
                nc.vector.tensor_scalar(
                    cwn[:, tb, :], cw, rcp[:, tb:tb + 1], None, op0=ALU.mult
                )
            nc.sync.dma_start(
                comb_shard[:].rearrange("(b p) e -> p b e", b=TBS), cwn
            )
            # replicate the combine table across cores (8KB -> 64KB)
            nc.gpsimd.collective_compute(
                "AllGather",
                ALU.bypass,
                replica_groups=[list(range(NCORE))],
                ins=[comb_shard[:]],
                outs=[comb_all[:]],
            )
            # ============ mm1 base: column shard, 4 double-buffered passes =
            ht_b = htp.tile([P, IC_B, NTOK], F16, tag="htb")
            for psx in range(NPASS):
                tsl = slice(psx * TPASS, (psx + 1) * TPASS)
                xt = xtp.tile([P, KO, TPASS], F16, tag="xt")
                nc.sync.dma_start(xt, xt16_d[:, :, tsl])
                for j in range(IC_B):
                    slab = wgup.tile(
                        [P, KO, 2 * P], F16, tag="slab", name=f"slb{psx}_{j}"
                    )
                    nc.sync.dma_start(slab, wgu_d[:, IC_E + j])
                    gsl = slice(psx * TPASS, (psx + 1) * TPASS)
                    pg = ps1.tile([P, NB1], F32, tag="pg", name=f"bpg{psx}_{j}")
                    pu = ps1.tile([P, NB1], F32, tag="pu", name=f"bpu{psx}_{j}")
                    for k in range(KO):
                        nc.tensor.matmul(
                            pg, slab[:, k, 0:P], xt[:, k, :],
                            start=(k == 0), stop=(k == KO - 1),
                        )
                    for k in range(KO):
                        nc.tensor.matmul(
                            pu, slab[:, k, P:2 * P], xt[:, k, :],
                            start=(k == 0), stop=(k == KO - 1),
                        )
                    sil = tmpp.tile([P, NB1], F32, tag="sil")
                    nc.scalar.activation(sil, pg, AF.Silu)
                    nc.vector.tensor_tensor(ht_b[:, j, gsl], sil, pu, ALU.mult)

            # ============ gather + transpose dispatched tokens ============
            xeT = persist.tile([P, KO, C], F16, tag="xeT")
            for t in range(NTC):
                xg = xgp.tile([P, H], F16, tag="xg")
                nc.vector.memset(xg, 0.0)
                nc.gpsimd.indirect_dma_start(
                    out=xg[:],
                    out_offset=None,
                    in_=xrow_d[:],
                    in_offset=bass.IndirectOffsetOnAxis(ap=idx_sb[:, t:t + 1], axis=0),
                    bounds_check=NTOK - 1,
                    oob_is_err=False,
                )
                for k in range(KO):
                    ptr = ps2.tile([P, P], F16, tag="py", bufs=3, name=f"ptr{t}_{k}")
                    nc.tensor.transpose(ptr, xg[:, k * P:(k + 1) * P], identf)
                    nc.vector.tensor_copy(xeT[:, k, t * P:(t + 1) * P], ptr)

            # ============ mm1 expert: gate/up + silu*up on C tokens ========
            ht_e = htp.tile([P, IC_E, C], F16, tag="hte")
            for i in range(IC_E):
                slab = wgup.tile([P, KO, 2 * P], F16, tag="slab", name=f"sl{i}")
                nc.sync.dma_start(slab, wgu_d[:, i])
                for (no, nw) in _chunks(C, NB1):
                    nsl = slice(no, no + nw)
                    pg = ps1.tile([P, NB1], F32, tag="pg", name=f"pg{i}_{no}")
                    pu = ps1.tile([P, NB1], F32, tag="pu", name=f"pu{i}_{no}")
                    for k in range(KO):
                        nc.tensor.matmul(
                            pg[:, :nw], slab[:, k, 0:P], xeT[:, k, nsl],
                            start=(k == 0), stop=(k == KO - 1),
                        )
                    for k in range(KO):
                        nc.tensor.matmul(
                            pu[:, :nw], slab[:, k, P:2 * P], xeT[:, k, nsl],
                            start=(k == 0), stop=(k == KO - 1),
                        )
                    sil = tmpp.tile([P, NB1], F32, tag="sil")
                    nc.scalar.activation(sil[:, :nw], pg[:, :nw], AF.Silu)
                    nc.vector.tensor_tensor(
                        ht_e[:, i, nsl], sil[:, :nw], pu[:, :nw], ALU.mult
                    )

            # per dispatched token: gather its comb row, dot with one-hot
            comb_g = persist.tile([P, NTC], F32, tag="combg")
            for t in range(NTC):
                cg = rmwp.tile([P, E], F32, tag="cg")
                nc.vector.memset(cg, 0.0)
                nc.gpsimd.indirect_dma_start(
                    out=cg[:],
                    out_offset=None,
                    in_=comb_all[:],
                    in_offset=bass.IndirectOffsetOnAxis(ap=idx_sb[:, t:t + 1], axis=0),
                    bounds_check=NTOK - 1,
                    oob_is_err=False,
                )
                cm = rmwp.tile([P, E], F32, tag="cm")
                nc.vector.tensor_tensor(cm, cg, esel_sb, ALU.mult)
                nc.vector.reduce_sum(comb_g[:, t:t + 1], cm, axis=AXX)

            # ============ mm2 (down) + combine, one column quarter at a time
            # Per quarter: base rows first (DMA writes overlap the expert
            # matmuls), then expert rows scaled by comb and scatter-ADDed
            # into the quarter's RS input via gpsimd accumulate DMA.
            # Phase A: ALL quarters' base down-proj first — every quarter's
            # dense rows reach DRAM before any expert work, so scatter-adds
            # and each RS fire with zero base-write wait (the per-quarter
            # SEMAPHORE_20 waits on the CC engine disappear).
            wd_base_all = persist.tile([P, HNC, IC_B, HN], F16, tag="wdb")
            for hn in range(HNC):
                nc.sync.dma_start(wd_base_all[:, hn], wd_d[:, hn, IC_E:ICT])
            wd_exp = {}
            for hn in range(2):
                wd_exp[hn] = wdp.tile(
                    [P, IC_E, HN], F16, tag="wsl", bufs=2, name=f"wde{hn}"
                )
                nc.sync.dma_start(wd_exp[hn], wd_d[:, hn, 0:IC_E])
            for hn in range(HNC):
                for tb in range(TBF):
                    py = ps2.tile(
                        [P, HN], F32, tag="py", bufs=3, name=f"pyb{hn}_{tb}"
                    )
                    for j in range(IC_B):
                        nc.tensor.matmul(
                            py, ht_b[:, j, tb * P:(tb + 1) * P],
                            wd_base_all[:, hn, j, :],
                            start=(j == 0), stop=(j == IC_B - 1),
                        )
                    yst = ystp.tile([P, HN], F16, tag="yst")
                    nc.vector.tensor_copy(yst, py)
                    nc.sync.dma_start(
                        rs_half[hn][tb * P:(tb + 1) * P, :],
                        yst,
                    )
            # Phase B: per quarter, expert down-proj with fused scale +
            # scatter-add, then the quarter's f16 ReduceScatter.
            scatters = []
            for half in range(NQ):
                hn = half
                if hn >= 2:
                    wd_exp[hn] = wdp.tile(
                        [P, IC_E, HN], F16, tag="wsl", bufs=2, name=f"wde{hn}"
                    )
                    nc.sync.dma_start(wd_exp[hn], wd_d[:, hn, 0:IC_E])
                last_scatter = None
                for t in range(NTC):
                    py = ps2.tile(
                        [P, HN], F32, tag="py", bufs=3, name=f"pye{hn}_{t}"
                    )
                    for i in range(IC_E):
                        nc.tensor.matmul(
                            py, ht_e[:, i, t * P:(t + 1) * P],
                            wd_exp[hn][:, i, :],
                            start=(i == 0), stop=(i == IC_E - 1),
                        )
                    sc = rmwp.tile([P, HQ], F16, tag="sc", bufs=6)
                    nc.vector.tensor_scalar_mul(sc, py, comb_g[:, t:t + 1])
                    last_scatter = nc.gpsimd.indirect_dma_start(
                        out=rs_half[half][:],
                        out_offset=bass.IndirectOffsetOnAxis(
                            ap=idx_sb[:, t:t + 1], axis=0
                        ),
                        in_=sc[:],
                        in_offset=None,
                        bounds_check=NTOK - 1,
                        oob_is_err=False,
                        compute_op=ALU.add,
                    )
                scatters.append(last_scatter)
                # combine across cores for this column half
                nc.gpsimd.collective_compute(
                    "ReduceScatter",
                    ALU.add,
                    replica_groups=[list(range(NCORE))],
                    ins=[rs_half[half][:]],
                    outs=[rs_out[half][:]],
                )
            # Output copies: each is pinned (explicit dep) behind the RMW
            # scatter two quarters later, so its RS-completion wait is
            # already satisfied when it reaches the DMA queue — otherwise
            # the scheduler hoists it and the pending wait head-of-line
            # blocks every later DMA sharing its completion lane.
            for half in range(NQ):
                dma = nc.sync.dma_start(out_d[half], rs_out[half][:])
                dep = scatters[min(half + 2, NQ - 1)]
                bass._add_dep_helper(
                    dma.ins, dep.ins, sync=True, reason="defer rs_out copy"
                )

    return nc


def _prep_inputs(x, gate_w, base_gate_up, base_down, expert_gate_up, expert_down):
    xf = np.ascontiguousarray(np.asarray(x, np.float32).reshape(NTOK, H))
    xT = np.ascontiguousarray(xf.reshape(NTOK, KO, P).transpose(2, 1, 0))
    xt16 = xT.astype(np.float16)
    xrow16 = xf.astype(np.float16)
    gwf = np.asarray(gate_w, np.float32)
    gwp = np.ascontiguousarray(gwf.reshape(KO, P, E).transpose(1, 0, 2))

    # host-side dispatch: which tokens go to which expert (top-2 of logits)
    logits = xf @ gwf
    order = np.argsort(-logits, axis=1)
    top2 = order[:, :2]
    sel = [np.where((top2 == c).any(axis=1))[0].astype(np.int32) for c in range(NCORE)]
    cmax = max(len(s) for s in sel)
    C = max(P, ((cmax + P - 1) // P) * P)

    SH = I_EXP // NCORE
    in_maps = []
    for c in range(NCORE):
        We = np.asarray(expert_gate_up[c], np.float32)
        ge_ = We[:, :I_EXP].reshape(H, IC_E, P)
        ue_ = We[:, I_EXP:].reshape(H, IC_E, P)
        pe_ = np.concatenate([ge_, ue_], axis=2)
        bgu = np.asarray(base_gate_up, np.float32)
        gb_ = bgu[:, c * SH:(c + 1) * SH].reshape(H, IC_B, P)
        ub_ = bgu[:, I_EXP + c * SH: I_EXP + (c + 1) * SH].reshape(H, IC_B, P)
        pb_ = np.concatenate([gb_, ub_], axis=2)
        allp = np.concatenate([pe_, pb_], axis=1)  # [H, ICT, 2P]
        wgu_p = np.ascontiguousarray(
            allp.reshape(KO, P, ICT, 2 * P).transpose(1, 2, 0, 3)
        ).astype(np.float16)
        wdcat = np.concatenate(
            [
                np.asarray(expert_down[c], np.float32),
                np.asarray(base_down, np.float32)[c * SH:(c + 1) * SH],
            ],
            axis=0,
        )
        wd_p = np.ascontiguousarray(
            wdcat.reshape(ICT, P, HNC, HN).transpose(1, 2, 0, 3)
        ).astype(np.float16)
        es = np.zeros((P, E), np.float32)
        es[:, c] = 1.0
        tix = np.full(C, OOB_IDX, np.int32)
        tix[: len(sel[c])] = sel[c]
        tix = np.ascontiguousarray(tix.reshape(C // P, P).T)
        xt32_shard = np.ascontiguousarray(xT[:, :, c * NSH:(c + 1) * NSH])
        in_maps.append(
            dict(
                xt16=xt16, xt32=xt32_shard, xrow=xrow16, wgu=wgu_p, wd=wd_p,
                gw=gwp, esel=es, tidx=tix,
            )
        )
    return in_maps, C


LAST_RESULTS = None


def kernel(x, gate_w, base_gate_up, base_down, expert_gate_up, expert_down):
    global LAST_RESULTS
    in_maps, C = _prep_inputs(
        x, gate_w, base_gate_up, base_down, expert_gate_up, expert_down
    )
    nc = _build(C)
    if not nc.is_finalized():
        nc.finalize()
    res = run_bass_kernel_spmd(nc, in_maps, core_ids=list(range(NCORE)))
    LAST_RESULTS = res
    y = np.empty((NTOK, H), np.float32)
    for c in range(NCORE):
        o = res.results[c]["out"]  # [NQ, 256, HQ] f16
        rows = slice(c * (NTOK // NCORE), (c + 1) * (NTOK // NCORE))
        for q in range(NQ):
            y[rows, q * HQ:(q + 1) * HQ] = o[q].astype(np.float32)
    return y.reshape(1, NTOK, H)


if __name__ == "__main__":
    nc = _build(640)
    print("build ok; instructions:", sum(len(b.instructions) for b in nc.main_func.blocks))


# revision 14
# speedup vs baseline: 1.1032x; 1.1032x over previous
"""LlamaMoE (H=2048, I=4096, E=8 experts, top-2, N=2048 tokens) on 8 trn2 cores.

Strategy: expert-parallel with sparse token dispatch. Core c owns expert c
and computes it only on the ~C tokens routed to it (host supplies the
dispatch permutation = token indices per expert, padded with OOB sentinels;
all model math — router logits, top-2 combine weights, expert MLPs, base
MLP, combine — runs on device). Base MLP is column-sharded 1/8 per core.

Router is token-sharded: each core computes fp32 logits + top-2 combine
weights for its 256-token shard (all 8 expert columns), then a small
AllGather replicates the full [2048, 8] combine table; each core extracts
its expert's column for its dispatched tokens via indirect gather + dot
with a one-hot. Expert rows are combined into the dense base rows with a
single scatter-ADD DMA (gpsimd software DGE accumulate) per token chunk.

Order: router -> base mm1 (4 double-buffered token passes) -> gather +
transpose dispatched tokens -> expert mm1 -> per column quarter (base
down first so its DRAM writes are in flight, then expert down with fused
scale + scatter-add, then f16 ReduceScatter). Each core returns a
disjoint f16 row shard which the host concatenates and casts.
"""

import numpy as np

import concourse.bacc as bacc
import concourse.bass as bass
import concourse.mybir as mybir
import concourse.tile as tile
from concourse.bass_utils import run_bass_kernel_spmd
from concourse.masks import make_identity

P = 128
H = 2048
I_EXP = 4096
E = 8
NCORE = 8
NTOK = 2048
NSH = NTOK // NCORE         # 256 router-shard tokens per core
TBS = NSH // P              # 2 router token blocks
KO = H // P                 # 16 contraction tiles for mm1
IC_E = I_EXP // P           # 32 expert intermediate chunks
IC_B = (I_EXP // NCORE) // P  # 4 base-shard chunks per core
ICT = IC_E + IC_B           # 36 contraction tiles for mm2
NPASS = 4                   # token passes for the base MLP
TPASS = NTOK // NPASS       # 512
NB1 = 512                   # mm1 moving free dim (tokens)
HN = 256                    # mm2 moving free dim (H cols)
HNC = H // HN               # 8
NQ = 8                      # column slices for combine/RS pipelining
HQ = H // NQ                # 256
TBF = NTOK // P             # 16 token blocks (full)

F32 = mybir.dt.float32
F16 = mybir.dt.float16
I32 = mybir.dt.int32
AF = mybir.ActivationFunctionType
ALU = mybir.AluOpType
AXX = mybir.AxisListType.X

OOB_IDX = 1 << 20


def _chunks(total, step):
    out = []
    o = 0
    while o < total:
        out.append((o, min(step, total - o)))
        o += step
    return out


def _build(C, LA):
    NTC = C // P  # gathered token chunks
    nc = bacc.Bacc(None)
    xt16_d = nc.dram_tensor("xt16", [P, KO, NTOK], F16, kind="ExternalInput")
    xt32_d = nc.dram_tensor("xt32", [P, KO, NSH], F32, kind="ExternalInput")
    xrow_d = nc.dram_tensor("xrow", [NTOK, H], F16, kind="ExternalInput")
    wgu_d = nc.dram_tensor("wgu", [P, ICT, KO, 2 * P], F16, kind="ExternalInput")
    wd_d = nc.dram_tensor("wd", [P, HNC, ICT, HN], F16, kind="ExternalInput")
    gw_d = nc.dram_tensor("gw", [P, KO, E], F32, kind="ExternalInput")
    esel_d = nc.dram_tensor("esel", [P, E], F32, kind="ExternalInput")
    tidx_d = nc.dram_tensor("tidx", [P, NTC], I32, kind="ExternalInput")
    out_d = nc.dram_tensor("out", [NQ, NTOK // NCORE, HQ], F16, kind="ExternalOutput")

    with tile.TileContext(nc) as tc:
        with (
            tc.tile_pool(name="persist", bufs=1) as persist,
            tc.tile_pool(name="xt", bufs=2) as xtp,
            tc.tile_pool(name="ht", bufs=1) as htp,
            tc.tile_pool(name="wgup", bufs=2) as wgup,
            tc.tile_pool(name="wdp", bufs=4) as wdp,
            tc.tile_pool(name="xk32", bufs=2) as xk32p,
            tc.tile_pool(name="xgp", bufs=3) as xgp,
            tc.tile_pool(name="tmp", bufs=3) as tmpp,
            tc.tile_pool(name="yst", bufs=6) as ystp,
            tc.tile_pool(name="rmw", bufs=3) as rmwp,
            tc.tile_pool(name="rsm", bufs=1) as rsm,
            tc.tile_pool(name="ps1", bufs=2, space="PSUM") as ps1,
            tc.tile_pool(name="ps2", bufs=2, space="PSUM") as ps2,
            tc.tile_pool(name="dram", bufs=1, space="DRAM") as dram,
        ):
            ident = persist.tile([P, P], F32, tag="ident")
            make_identity(nc, ident)
            identf = persist.tile([P, P], F16, tag="identf")
            make_identity(nc, identf)
            gw_sb = persist.tile([P, KO, E], F32, tag="gw")
            nc.sync.dma_start(gw_sb, gw_d[:])
            esel_sb = persist.tile([P, E], F32, tag="esel")
            nc.sync.dma_start(esel_sb, esel_d[:])
            idx_sb = persist.tile([P, NTC], I32, tag="idx")
            nc.sync.dma_start(idx_sb, tidx_d[:])

            rs_half = [
                dram.tile([NTOK, HQ], F16, tag=f"rsin{h_}", name=f"rsin{h_}")
                for h_ in range(NQ)
            ]
            rs_out = [
                dram.tile(
                    [NTOK // NCORE, HQ], F16, tag=f"rsout{h_}", name=f"rsout{h_}"
                )
                for h_ in range(NQ)
            ]
            comb_shard = dram.tile([NSH, E], F32, tag="combsh", name="combsh")
            comb_all = dram.tile([NTOK, E], F32, tag="comball", name="comball")

            # ============ router on this core's 256-token shard (fp32) =====
            lg_ps = ps1.tile([E, NSH], F32, tag="pg", name="lg")
            for k in range(KO):
                xk = xk32p.tile([P, NSH], F32, tag="xk")
                nc.sync.dma_start(xk, xt32_d[:, k, :])
                nc.tensor.matmul(
                    lg_ps, gw_sb[:, k, :], xk,
                    start=(k == 0), stop=(k == KO - 1),
                )
            lgt = rsm.tile([E, NSH], F32, tag="lgt")
            nc.vector.tensor_copy(lgt, lg_ps)
            zl = rsm.tile([P, TBS, E], F32, tag="zl")
            for tb in range(TBS):
                pt = ps2.tile([P, HN], F32, tag="py", bufs=3, name=f"pt{tb}")
                nc.tensor.transpose(
                    pt[:, :E], lgt[:, tb * P:(tb + 1) * P], ident[:E, :E]
                )
                nc.vector.tensor_copy(zl[:, tb, :], pt[:, :E])
            lmax = rsm.tile([P, TBS], F32, tag="lmax")
            nc.vector.reduce_max(lmax[:, :, None], zl, axis=AXX)
            nmax = rsm.tile([P, TBS], F32, tag="nmax")
            nc.vector.tensor_scalar_mul(nmax, lmax, -1.0)
            zex = rsm.tile([P, TBS, E], F32, tag="zex")
            for tb in range(TBS):
                nc.scalar.activation(
                    zex[:, tb, :], zl[:, tb, :], AF.Exp, bias=nmax[:, tb:tb + 1]
                )
            zlt = rsm.tile([P, TBS, E], F32, tag="zlt")
            nc.vector.tensor_scalar(zlt, zex, 1.0, None, op0=ALU.is_lt)
            zmk = rsm.tile([P, TBS, E], F32, tag="zmk")
            nc.vector.tensor_tensor(zmk, zex, zlt, ALU.mult)
            m2 = rsm.tile([P, TBS], F32, tag="m2")
            nc.vector.reduce_max(m2[:, :, None], zmk, axis=AXX)
            s1 = rsm.tile([P, TBS], F32, tag="s1")
            nc.vector.tensor_scalar_add(s1, m2, 1.0)
            rcp = rsm.tile([P, TBS], F32, tag="rcp")
            nc.vector.reciprocal(rcp, s1)
            # combine weight for every expert: zex * (zex >= m2) / (1 + m2)
            cwn = rsm.tile([P, TBS, E], F32, tag="cwn")
            for tb in range(TBS):
                ge = rsm.tile([P, E], F32, tag="ge")
                nc.vector.tensor_scalar(
                    ge, zex[:, tb, :], m2[:, tb:tb + 1], None, op0=ALU.is_ge
                )
                cw = rsm.tile([P, E], F32, tag="cw")
                nc.vector.tensor_tensor(cw, zex[:, tb, :], ge, ALU.mult)
                nc.vector.tensor_scalar(
                    cwn[:, tb, :], cw, rcp[:, tb:tb + 1], None, op0=ALU.mult
                )
            nc.sync.dma_start(
                comb_shard[:].rearrange("(b p) e -> p b e", b=TBS), cwn
            )
            # replicate the combine table across cores (8KB -> 64KB)
            nc.gpsimd.collective_compute(
                "AllGather",
                ALU.bypass,
                replica_groups=[list(range(NCORE))],
                ins=[comb_shard[:]],
                outs=[comb_all[:]],
            )
            # ============ mm1 base: column shard, 4 double-buffered passes =
            ht_b = htp.tile([P, IC_B, NTOK], F16, tag="htb")
            for psx in range(NPASS):
                tsl = slice(psx * TPASS, (psx + 1) * TPASS)
                xt = xtp.tile([P, KO, TPASS], F16, tag="xt")
                nc.sync.dma_start(xt, xt16_d[:, :, tsl])
                for j in range(IC_B):
                    slab = wgup.tile(
                        [P, KO, 2 * P], F16, tag="slab", name=f"slb{psx}_{j}"
                    )
                    nc.sync.dma_start(slab, wgu_d[:, IC_E + j])
                    gsl = slice(psx * TPASS, (psx + 1) * TPASS)
                    pg = ps1.tile([P, NB1], F32, tag="pg", name=f"bpg{psx}_{j}")
                    pu = ps1.tile([P, NB1], F32, tag="pu", name=f"bpu{psx}_{j}")
                    for k in range(KO):
                        nc.tensor.matmul(
                            pg, slab[:, k, 0:P], xt[:, k, :],
                            start=(k == 0), stop=(k == KO - 1),
                        )
                    for k in range(KO):
                        nc.tensor.matmul(
                            pu, slab[:, k, P:2 * P], xt[:, k, :],
                            start=(k == 0), stop=(k == KO - 1),
                        )
                    sil = tmpp.tile([P, NB1], F32, tag="sil")
                    nc.scalar.activation(sil, pg, AF.Silu)
                    nc.vector.tensor_tensor(ht_b[:, j, gsl], sil, pu, ALU.mult)

            # ============ gather + transpose dispatched tokens ============
            xeT = persist.tile([P, KO, C], F16, tag="xeT")
            for t in range(NTC):
                xg = xgp.tile([P, H], F16, tag="xg")
                nc.vector.memset(xg, 0.0)
                nc.gpsimd.indirect_dma_start(
                    out=xg[:],
                    out_offset=None,
                    in_=xrow_d[:],
                    in_offset=bass.IndirectOffsetOnAxis(ap=idx_sb[:, t:t + 1], axis=0),
                    bounds_check=NTOK - 1,
                    oob_is_err=False,
                )
                for k in range(KO):
                    ptr = ps2.tile([P, P], F16, tag="py", bufs=3, name=f"ptr{t}_{k}")
                    nc.tensor.transpose(ptr, xg[:, k * P:(k + 1) * P], identf)
                    nc.vector.tensor_copy(xeT[:, k, t * P:(t + 1) * P], ptr)

            # ============ mm1 expert: gate/up + silu*up on LA tokens =======
            # Only the first LA (true max expert load) token columns get
            # computed; the C-LA pad columns are zeroed once so mm2 reads
            # well-defined values (their output rows are OOB-dropped).
            ht_e = htp.tile([P, IC_E, C], F16, tag="hte")
            if LA < C:
                nc.vector.memset(ht_e[:, :, LA:C], 0.0)
            for i in range(IC_E):
                slab = wgup.tile([P, KO, 2 * P], F16, tag="slab", name=f"sl{i}")
                nc.sync.dma_start(slab, wgu_d[:, i])
                for (no, nw) in _chunks(LA, NB1):
                    nsl = slice(no, no + nw)
                    pg = ps1.tile([P, NB1], F32, tag="pg", name=f"pg{i}_{no}")
                    pu = ps1.tile([P, NB1], F32, tag="pu", name=f"pu{i}_{no}")
                    for k in range(KO):
                        nc.tensor.matmul(
                            pg[:, :nw], slab[:, k, 0:P], xeT[:, k, nsl],
                            start=(k == 0), stop=(k == KO - 1),
                        )
                    for k in range(KO):
                        nc.tensor.matmul(
                            pu[:, :nw], slab[:, k, P:2 * P], xeT[:, k, nsl],
                            start=(k == 0), stop=(k == KO - 1),
                        )
                    sil = tmpp.tile([P, NB1], F32, tag="sil")
                    nc.scalar.activation(sil[:, :nw], pg[:, :nw], AF.Silu)
                    nc.vector.tensor_tensor(
                        ht_e[:, i, nsl], sil[:, :nw], pu[:, :nw], ALU.mult
                    )

            # per dispatched token: gather its comb row, dot with one-hot
            comb_g = persist.tile([P, NTC], F32, tag="combg")
            for t in range(NTC):
                cg = rmwp.tile([P, E], F32, tag="cg")
                nc.vector.memset(cg, 0.0)
                nc.gpsimd.indirect_dma_start(
                    out=cg[:],
                    out_offset=None,
                    in_=comb_all[:],
                    in_offset=bass.IndirectOffsetOnAxis(ap=idx_sb[:, t:t + 1], axis=0),
                    bounds_check=NTOK - 1,
                    oob_is_err=False,
                )
                cm = rmwp.tile([P, E], F32, tag="cm")
                nc.vector.tensor_tensor(cm, cg, esel_sb, ALU.mult)
                nc.vector.reduce_sum(comb_g[:, t:t + 1], cm, axis=AXX)

            # ============ mm2 (down) + combine, one column quarter at a time
            # Per quarter: base rows first (DMA writes overlap the expert
            # matmuls), then expert rows scaled by comb and scatter-ADDed
            # into the quarter's RS input via gpsimd accumulate DMA.
            scatters = []
            for half in range(NQ):
                for hh in range(HNC // NQ):
                    hn = half * (HNC // NQ) + hh
                    ICH = ICT // 2  # 18: slabs halved to fit SBUF
                    wslA = wdp.tile([P, ICH, HN], F16, tag="wsl", name=f"wslA{hn}")
                    nc.sync.dma_start(wslA, wd_d[:, hn, 0:ICH])
                    wslB = wdp.tile([P, ICH, HN], F16, tag="wsl", name=f"wslB{hn}")
                    nc.sync.dma_start(wslB, wd_d[:, hn, ICH:ICT])

                    def wsl_i(i):
                        return wslA[:, i, :] if i < ICH else wslB[:, i - ICH, :]

                    # base down on all tokens -> dense rows of rs_half
                    for tb in range(TBF):
                        py = ps2.tile(
                            [P, HN], F32, tag="py", bufs=3, name=f"pyb{hn}_{tb}"
                        )
                        for j in range(IC_B):
                            nc.tensor.matmul(
                                py, ht_b[:, j, tb * P:(tb + 1) * P],
                                wsl_i(IC_E + j),
                                start=(j == 0), stop=(j == IC_B - 1),
                            )
                        yst = ystp.tile([P, HN], F16, tag="yst")
                        nc.vector.tensor_copy(yst, py)
                        nc.sync.dma_start(
                            rs_half[half][tb * P:(tb + 1) * P,
                                          hh * HN:(hh + 1) * HN],
                            yst,
                        )
                    # expert down on gathered tokens: scale + scatter-add
                    last_scatter = None
                    for t in range(NTC):
                        py = ps2.tile(
                            [P, HN], F32, tag="py", bufs=3, name=f"pye{hn}_{t}"
                        )
                        for i in range(IC_E):
                            nc.tensor.matmul(
                                py, ht_e[:, i, t * P:(t + 1) * P], wsl_i(i),
                                start=(i == 0), stop=(i == IC_E - 1),
                            )
                        sc = rmwp.tile([P, HQ], F16, tag="sc", bufs=6)
                        nc.vector.tensor_scalar_mul(sc, py, comb_g[:, t:t + 1])
                        last_scatter = nc.gpsimd.indirect_dma_start(
                            out=rs_half[half][:],
                            out_offset=bass.IndirectOffsetOnAxis(
                                ap=idx_sb[:, t:t + 1], axis=0
                            ),
                            in_=sc[:],
                            in_offset=None,
                            bounds_check=NTOK - 1,
                            oob_is_err=False,
                            compute_op=ALU.add,
                        )
                scatters.append(last_scatter)
                # combine across cores for this column half
                nc.gpsimd.collective_compute(
                    "ReduceScatter",
                    ALU.add,
                    replica_groups=[list(range(NCORE))],
                    ins=[rs_half[half][:]],
                    outs=[rs_out[half][:]],
                )
            # Output copies: each is pinned (explicit dep) behind the RMW
            # scatter two quarters later, so its RS-completion wait is
            # already satisfied when it reaches the DMA queue — otherwise
            # the scheduler hoists it and the pending wait head-of-line
            # blocks every later DMA sharing its completion lane.
            for half in range(NQ):
                dma = nc.sync.dma_start(out_d[half], rs_out[half][:])
                dep = scatters[min(half + 2, NQ - 1)]
                bass._add_dep_helper(
                    dma.ins, dep.ins, sync=True, reason="defer rs_out copy"
                )

    return nc


def _prep_inputs(x, gate_w, base_gate_up, base_down, expert_gate_up, expert_down):
    xf = np.ascontiguousarray(np.asarray(x, np.float32).reshape(NTOK, H))
    xT = np.ascontiguousarray(xf.reshape(NTOK, KO, P).transpose(2, 1, 0))
    xt16 = xT.astype(np.float16)
    xrow16 = xf.astype(np.float16)
    gwf = np.asarray(gate_w, np.float32)
    gwp = np.ascontiguousarray(gwf.reshape(KO, P, E).transpose(1, 0, 2))

    # host-side dispatch: which tokens go to which expert (top-2 of logits)
    logits = xf @ gwf
    order = np.argsort(-logits, axis=1)
    top2 = order[:, :2]
    sel = [np.where((top2 == c).any(axis=1))[0].astype(np.int32) for c in range(NCORE)]
    cmax = max(len(s) for s in sel)
    C = max(P, ((cmax + P - 1) // P) * P)
    LA = cmax

    SH = I_EXP // NCORE
    in_maps = []
    for c in range(NCORE):
        We = np.asarray(expert_gate_up[c], np.float32)
        ge_ = We[:, :I_EXP].reshape(H, IC_E, P)
        ue_ = We[:, I_EXP:].reshape(H, IC_E, P)
        pe_ = np.concatenate([ge_, ue_], axis=2)
        bgu = np.asarray(base_gate_up, np.float32)
        gb_ = bgu[:, c * SH:(c + 1) * SH].reshape(H, IC_B, P)
        ub_ = bgu[:, I_EXP + c * SH: I_EXP + (c + 1) * SH].reshape(H, IC_B, P)
        pb_ = np.concatenate([gb_, ub_], axis=2)
        allp = np.concatenate([pe_, pb_], axis=1)  # [H, ICT, 2P]
        wgu_p = np.ascontiguousarray(
            allp.reshape(KO, P, ICT, 2 * P).transpose(1, 2, 0, 3)
        ).astype(np.float16)
        wdcat = np.concatenate(
            [
                np.asarray(expert_down[c], np.float32),
                np.asarray(base_down, np.float32)[c * SH:(c + 1) * SH],
            ],
            axis=0,
        )
        wd_p = np.ascontiguousarray(
            wdcat.reshape(ICT, P, HNC, HN).transpose(1, 2, 0, 3)
        ).astype(np.float16)
        es = np.zeros((P, E), np.float32)
        es[:, c] = 1.0
        tix = np.full(C, OOB_IDX, np.int32)
        tix[: len(sel[c])] = sel[c]
        tix = np.ascontiguousarray(tix.reshape(C // P, P).T)
        xt32_shard = np.ascontiguousarray(xT[:, :, c * NSH:(c + 1) * NSH])
        in_maps.append(
            dict(
                xt16=xt16, xt32=xt32_shard, xrow=xrow16, wgu=wgu_p, wd=wd_p,
                gw=gwp, esel=es, tidx=tix,
            )
        )
    return in_maps, C, LA


LAST_RESULTS = None


def kernel(x, gate_w, base_gate_up, base_down, expert_gate_up, expert_down):
    global LAST_RESULTS
    in_maps, C, LA = _prep_inputs(
        x, gate_w, base_gate_up, base_down, expert_gate_up, expert_down
    )
    nc = _build(C, LA)
    if not nc.is_finalized():
        nc.finalize()
    res = run_bass_kernel_spmd(nc, in_maps, core_ids=list(range(NCORE)))
    LAST_RESULTS = res
    y = np.empty((NTOK, H), np.float32)
    for c in range(NCORE):
        o = res.results[c]["out"]  # [NQ, 256, HQ] f16
        rows = slice(c * (NTOK // NCORE), (c + 1) * (NTOK // NCORE))
        for q in range(NQ):
            y[rows, q * HQ:(q + 1) * HQ] = o[q].astype(np.float32)
    return y.reshape(1, NTOK, H)


if __name__ == "__main__":
    nc = _build(640, 545)
    print("build ok; instructions:", sum(len(b.instructions) for b in nc.main_func.blocks))
